# revision 27
# baseline (speedup 1.0000x reference)
"""Trainium2 Bass kernel for nn_Attention_33638183862624 (linear/Taylor-softmax
attention). Data-parallel over batch: 16 batches -> 8 NeuronCores, 2 each.

Math per batch (C=512, N=4096, CQK=64), x flattened to [C, N]:
  Q = Wq x + bq; K = Wk x + bk           (Q,K: [64, N])
  Qn = Q / ||Q||_col; Kn = K / ||K||_col
  ksum = sum_n Kn[:, n]                  [64]
  denom = N + Qn^T ksum; tailor = 1/denom
  V = Wv x + bv                          (never materialized; fused:)
  matrix = Kn V^T = (Kn x^T) Wv^T + ksum bv^T            [64, 512]
  vsum   = V 1_N  = Wv (x 1_N) + N bv                    [512]
  out[c,n] = gamma * tailor[n] * (vsum[c] + sum_m matrix[m,c] Qn[m,n])

v3 design (fp16, deep software pipeline):
  * All IO fp16; host pre-transposes x^T and lays x/x^T/out out so every DMA
    reads/writes 4KB-contiguous per partition (good SDMA descriptors).
  * All matmuls fp16 (FWL halves LDWEIGHTS; fp32 PSUM accumulate).
  * The PE instruction stream is kept dense (HAM stays at K=8/8): every PE
    consumer trails its cross-engine producer by 1-2 chunks:
      chunk nb emits: QK(nb) | P(nb-2) | n2q8(nb-1) | ktT(nb-1) |
                      t2(b-1) | out(b-1, nb-1)
    and A2 (batch finalization) interleaves with the next batch's chunks.
  * Q stays raw; with s = ksum^T Q_raw and nq = ||Q||_col:
      un := N*u = N/(N*nq + s), and N*tailor = nq*un;
    1/N folded into mat_sb, gamma applied in the final PSUM->SBUF drain.
  * ks8 (the stacked-ksum lhsT for s8) built by one outer-product matmul
    instead of 8 DVE copies.
"""

import numpy as np

B, C, H, W = 16, 512, 64, 64
N = H * W          # 4096
CQK = C // 8       # 64
NCORES = 8
BLOC = B // NCORES  # 2 batches per core
NB = N // 512       # 8 n-chunks of 512
KC = C // 128       # 4 channel chunks of 128


# ---------------------------------------------------------------------------
# Walrus workaround: this container's walrus rejects >1 sync wait per
# instruction ("Too many sync wait commands"). (1) patch the TileContext tail
# drain to carry its waits on single-wait NOPs; (2) post-pass that rewrites
# any instruction with k>1 waits into k-1 single-wait NOPs + the instruction.
# ---------------------------------------------------------------------------

def _apply_tile_patches():
    import concourse.tile as tile
    from concourse import mybir
    from concourse.vector_clock import ScopedClock

    if getattr(tile.TileContext, "_drain_patched", False):
        return

    def _patched_drain_and_barrier(self, tick_clock, wait_clock):
        nop = self.nc.sync.nop(nofuse=True, hint="tail_drain_waits")
        wait_clock.add_sem_waits(
            nop.ins, ScopedClock({None: tick_clock.global_clock})
        )
        si = nop.ins.sync_info
        if si is not None and len(si.on_wait) > 1:
            waits = list(si.on_wait)
            nop.ins.sync_info = mybir.SyncInfo(on_wait=waits[:1], on_update=[])
            rest = waits[1:]
            while rest:
                n2 = self.nc.sync.nop(nofuse=True, hint="tail_drain_waits")
                n2.ins.sync_info = mybir.SyncInfo(on_wait=rest[:1], on_update=[])
                rest = rest[1:]
        self.nc.sync.drain()
        self.nc.all_engine_barrier()
        assert self.sems is not None
        popped = self.nc._tile_sem_poison_stack.pop()
        assert popped is self._sem_poison
        self.nc.clear_and_free_semaphores(list(self.sems.allocated().values()))
        self.nc.all_engine_barrier()

    tile.TileContext._drain_and_barrier = _patched_drain_and_barrier
    tile.TileContext._drain_patched = True


def _split_multi_waits(nc):
    from concourse import mybir

    counter = [0]
    for f in nc.m.functions:
        for bb in f.blocks:
            insts = bb.instructions
            if not any(
                i.sync_info is not None and len(i.sync_info.on_wait) > 1
                for i in insts
            ):
                continue
            new = []
            for ins in insts:
                si = ins.sync_info
                if si is not None and len(si.on_wait) > 1:
                    waits = list(si.on_wait)
                    for w in waits[:-1]:
                        counter[0] += 1
                        nop = mybir.InstNoOp(
                            name=f"I-wsplit-{counter[0]}", ins=[], outs=[]
                        )
                        nop.engine = ins.engine
                        nop.sync_info = mybir.SyncInfo(on_wait=[w], on_update=[])
                        new.append(nop)
                    ins.sync_info = mybir.SyncInfo(
                        on_wait=[waits[-1]], on_update=list(si.on_update)
                    )
                new.append(ins)
            bb.instructions = new


# ---------------------------------------------------------------------------
# Kernel body
# ---------------------------------------------------------------------------

# const blob column layout (fp16, [128, CBLOB]); wqkt/wvt lead so one DMA
# brings everything in with 4.4KB-contiguous per-partition descriptors
CB_WQKT = 0           # [128, 4*128] Wqk^T chunks
CB_WVT = 512          # [128, 4*512] Wv^T chunks
CB_IDENT = 2560       # [128, 128] identity
CB_SEL8 = 2688        # [64, 64] n2q8 one-hot stacker
CB_SELAB = 2752       # [40, 520] u/tailor broadcast selector
CB_ONES2 = 3272       # [128, 2] ones
CB_ONESN = 3274       # [1, 512] ones row
CB_BQK = 3786         # [1, 128] qk bias row
CB_BV = 3914          # [1, 512] v bias row
CB_KPAT = 4426        # [1, 64] ks8 diagonal pattern row
CBLOB = 4490


def _build_module(use_bqk=True, use_bv=True):
    import concourse.bass as bass
    import concourse.tile as tile
    from concourse import mybir

    _apply_tile_patches()
    f16 = mybir.dt.float16
    f32 = mybir.dt.float32
    alu = mybir.AluOpType
    act = mybir.ActivationFunctionType

    nc = bass.Bass("TRN2", target_bir_lowering=False, debug=False)

    # x5:  [b, nb, p, k, 512]  (c = 128k + p)
    # xt5: [b, nb, p, j, 512]  (n = 512nb + 128j + p)
    # o5:  [b, nb, p, cb, 512] (c = 128cb + p)
    x_d = nc.dram_tensor("x5", [BLOC, NB, 128, KC, 512], f16,
                         kind="ExternalInput").ap()
    xt_d = nc.dram_tensor("xt5", [BLOC, NB, 128, 4, 512], f16,
                          kind="ExternalInput").ap()
    blob_d = nc.dram_tensor("blob", [128, CBLOB], f16, kind="ExternalInput").ap()
    gam_d = nc.dram_tensor("gamma", [1, 1], f32, kind="ExternalInput").ap()
    out_d = nc.dram_tensor("out", [BLOC, NB, 128, KC, 512], f16,
                           kind="ExternalOutput").ap()

    from contextlib import ExitStack

    with tile.TileContext(nc) as tc, ExitStack() as ctx, \
            nc.allow_low_precision(reason="fp16 compute validated vs 2e-2 gate"):
        consts = ctx.enter_context(tc.tile_pool(name="consts", bufs=1))
        xpool = ctx.enter_context(tc.tile_pool(name="xpool", bufs=3))
        xtpool = ctx.enter_context(tc.tile_pool(name="xtpool", bufs=5))
        batchp = ctx.enter_context(tc.tile_pool(name="batchp", bufs=2))
        work = ctx.enter_context(tc.tile_pool(name="work", bufs=3))
        outp = ctx.enter_context(tc.tile_pool(name="outp", bufs=6))
        pp_big = ctx.enter_context(tc.tile_pool(name="pp_big", bufs=5, space="PSUM"))
        pp_acc = ctx.enter_context(tc.tile_pool(name="pp_acc", bufs=2, space="PSUM"))
        pp_small = ctx.enter_context(
            tc.tile_pool(name="pp_small", bufs=1, space="PSUM")
        )

        # ---- constants ----
        # wqkt + misc land before the first x loads; the big wvt block is
        # deferred (emitted at batch 0 slot 1) since it's first read at A2a
        blob = consts.tile([128, CBLOB], f16)
        nc.sync.dma_start(out=blob[:, 0:CB_WVT], in_=blob_d[:, 0:CB_WVT])
        gam128 = consts.tile([128, 1], f32)
        nc.sync.dma_start(
            out=gam128,
            in_=bass.AP(
                tensor=gam_d.tensor, offset=gam_d.offset,
                ap=[[0, 128], [1, 1]],
            ),
        )
        wqkt = blob[:, CB_WQKT:CB_WQKT + 512].rearrange(
            "p (k m) -> p k m", m=128)
        wvt = blob[:, CB_WVT:CB_WVT + 2048].rearrange(
            "p (k m) -> p k m", m=512)
        ident = blob[:, CB_IDENT:CB_IDENT + 128]
        sel8 = blob[0:64, CB_SEL8:CB_SEL8 + 64]
        selab = blob[0:40, CB_SELAB:CB_SELAB + 520]
        ones2 = blob[:, CB_ONES2:CB_ONES2 + 2]
        onesn = blob[0:1, CB_ONESN:CB_ONESN + 512]
        bqk = blob[0:1, CB_BQK:CB_BQK + 128]
        bv = blob[0:1, CB_BV:CB_BV + 512]
        kpat = blob[0:1, CB_KPAT:CB_KPAT + 64]

        def alloc_state(b):
            st = {}
            st["q_raw"] = batchp.tile([65, N], f16, tag="q_raw",
                                      name=f"q_raw{b}")
            nc.gpsimd.memset(st["q_raw"][64:65, :], 1.0)
            st["ks8"] = batchp.tile([64, 64], f16, tag="ks8", name=f"ks8_{b}")
            st["ks_parts"] = batchp.tile([65, NB], f32, tag="ks_parts",
                                         name=f"ks_parts{b}")
            st["ksum_h"] = batchp.tile([65, 1], f16, tag="ksum_h",
                                       name=f"ksum_h{b}")
            st["ksumn_row"] = batchp.tile([1, 65], f16, tag="ksumn_row",
                                          name=f"ksumn_row{b}")
            st["p_sb"] = batchp.tile([65, 512], f16, tag="p_sb",
                                     name=f"p_sb{b}")
            st["pt_sb"] = batchp.tile([128, KC, 65], f16, tag="pt_sb",
                                      name=f"pt_sb{b}")
            st["mat_sb"] = batchp.tile([65, 512], f16, tag="mat_sb",
                                       name=f"mat_sb{b}")
            st["ut"] = batchp.tile([40, 512], f16, tag="ut", name=f"ut{b}")
            # rows 8-31 feed the t2 matmul with zero selab weights; stale
            # fp16 NaN patterns would still poison 0*NaN -> zero them.
            nc.gpsimd.memset(st["ut"][0:32, :], 0.0)
            st["nq8"] = batchp.tile([8, 512], f32, tag="nq8", name=f"nq8_{b}")
            st["t1"] = batchp.tile([8, 512], f32, tag="t1", name=f"t1_{b}")
            # persistent 3-deep ring for normalized K^T tiles: the ones
            # column is written once per ring slot instead of per chunk
            # (keeps gpsimd out of the knt critical chain)
            st["knt_ring"] = batchp.tile([128, 3, 4, 65], f16, tag="knt_ring",
                                         name=f"knt_ring{b}")
            nc.gpsimd.memset(st["knt_ring"][:, :, :, 64:65], 1.0)
            st["p_ps"] = None      # lazy (pp_acc)
            st["n2q8_ps"] = None   # lazy (pp_acc)
            st["xh"] = {}
            st["xt"] = {}
            st["k_sb"] = {}
            st["sq"] = {}
            st["knt"] = {}
            st["qns"] = {}
            return st

        # ---------------- emission pieces ----------------

        def emit_loads(b, st, nb, swdge=False):
            eng = nc.gpsimd if swdge else nc.sync
            xh = xpool.tile([128, KC, 512], f16, tag="xh", name=f"xh{b}_{nb}")
            eng.dma_start(out=xh, in_=x_d[b, nb])
            st["xh"][nb] = xh
            xt = xtpool.tile([128, 4, 512], f16, tag="xt", name=f"xt{b}_{nb}")
            eng.dma_start(out=xt, in_=xt_d[b, nb])
            st["xt"][nb] = xt

        def emit_qk_mm(b, st, nb):
            xh = st["xh"].pop(nb)
            qk_ps = pp_big.tile([128, 512], f32, tag="big", name=f"qk{b}_{nb}")
            for k in range(KC):
                nc.tensor.matmul(
                    qk_ps, wqkt[:, k, :], xh[:, k, :],
                    start=(k == 0), stop=(k == KC - 1 and not use_bqk),
                )
            if use_bqk:
                nc.tensor.matmul(qk_ps, bqk, onesn, start=False, stop=True)
            st["qk_ps"] = qk_ps

        def emit_qk_drain(b, st, nb):
            qk_ps = st.pop("qk_ps")
            sl = slice(512 * nb, 512 * (nb + 1))
            nc.vector.tensor_copy(out=st["q_raw"][0:64, sl], in_=qk_ps[0:64, :])
            sq_sb = work.tile([64, 512], f16, tag="sq", bufs=3,
                              name=f"sq{b}_{nb}")
            nc.scalar.square(out=sq_sb, in_=qk_ps[0:64, :])
            st["sq"][nb] = sq_sb
            k_sb = work.tile([64, 512], f16, tag="k_sb", bufs=3,
                             name=f"k_sb{b}_{nb}")
            nc.scalar.copy(out=k_sb, in_=qk_ps[64:128, :])
            st["k_sb"][nb] = k_sb

        def emit_n2q8_mm(b, st, nb):
            sq_sb = st["sq"].pop(nb)
            if st["n2q8_ps"] is None:
                st["n2q8_ps"] = pp_acc.tile([8, 512], f32, tag="acc",
                                            name=f"n2q8_ps{b}")
            nc.tensor.matmul(
                st["n2q8_ps"], sel8[:, 8 * nb:8 * (nb + 1)], sq_sb,
                start=(nb == 0), stop=(nb == NB - 1), skip_group_check=True,
            )

        def emit_ktT_mm(b, st, nb):
            k_sb = st["k_sb"].pop(nb)
            kt_ps = pp_big.tile([128, 4, 64], f16, tag="big",
                                name=f"kt{b}_{nb}")
            for j in range(4):
                nc.tensor.transpose(
                    kt_ps[:, j, :], k_sb[:, 128 * j:128 * (j + 1)],
                    ident[0:64, 0:64],
                )
            st["kt_ps"] = kt_ps

        def emit_knt_chain(b, st, nb):
            kt_ps = st.pop("kt_ps")
            # sum K^2 along the free dim of the transposed tiles:
            # ACT squares straight out of PSUM, one DVE reduce for all 4 j
            ksq = work.tile([128, 4, 64], f16, tag="ksq", bufs=2,
                            name=f"ksq{b}_{nb}")
            nc.scalar.square(out=ksq, in_=kt_ps)
            kn2 = work.tile([128, 4, 1], f32, tag="kn2", bufs=2,
                            name=f"kn2_{b}_{nb}")
            nc.vector.tensor_reduce(
                out=kn2, in_=ksq, axis=mybir.AxisListType.X, op=alu.add,
            )
            nkt = work.tile([128, 4], f32, tag="nkt", bufs=2,
                            name=f"nkt{b}_{nb}")
            nc.scalar.sqrt(out=nkt, in_=kn2[:, :, 0])
            rk = work.tile([128, 4], f32, tag="rk", bufs=2,
                           name=f"rk{b}_{nb}")
            nc.vector.reciprocal(out=rk, in_=nkt)
            knt_sb = st["knt_ring"][:, nb % 3]
            for j in range(4):
                nc.vector.tensor_scalar_mul(
                    out=knt_sb[:, j, 0:64], in0=kt_ps[:, j, :],
                    scalar1=rk[:, j:j + 1],
                )
            st["knt"][nb] = knt_sb
            # ksum partials need only knt -- run a slot ahead of the P stage
            ks_ps = pp_small.tile([65, 2], f32, tag="small", name=f"ksp{b}_{nb}")
            for j in range(4):
                nc.tensor.matmul(
                    ks_ps, knt_sb[:, j, :], ones2,
                    start=(j == 0), stop=(j == 3),
                    skip_group_check=True,
                )
            nc.vector.tensor_copy(
                out=st["ks_parts"][:, nb:nb + 1], in_=ks_ps[:, 0:1]
            )

        def emit_p_mm(b, st, nb):
            knt_sb = st["knt"].pop(nb)
            xt = st["xt"].pop(nb)
            if st["p_ps"] is None:
                st["p_ps"] = pp_acc.tile([65, 512], f32, tag="acc",
                                         name=f"p_ps{b}")
            for j in range(4):
                nc.tensor.matmul(
                    st["p_ps"], knt_sb[:, j, :], xt[:, j, :],
                    start=(nb == 0 and j == 0),
                    stop=(nb == NB - 1 and j == 3),
                    skip_group_check=True,
                )

        def emit_A2a1(b, st):
            # ksum chain + ks8 outer product (depends only on ks_parts)
            nc.vector.reduce_sum(
                out=st["ksum_h"], in_=st["ks_parts"],
                axis=mybir.AxisListType.X,
            )
            ksr_ps = pp_small.tile([1, 66], f32, tag="small", name=f"ksr{b}")
            nc.tensor.matmul(
                ksr_ps, st["ksum_h"], ident[0:65, 0:66], start=True, stop=True,
            )
            nc.vector.tensor_copy(out=st["ksumn_row"], in_=ksr_ps[0:1, 0:65])
            # ks8 = ksum (x) kpat : [64, 64] in one matmul
            ks8_ps = pp_small.tile([64, 64], f32, tag="small", name=f"ks8p{b}")
            nc.tensor.matmul(
                ks8_ps, st["ksumn_row"][0:1, 0:64], kpat, start=True, stop=True,
            )
            nc.scalar.copy(out=st["ks8"], in_=ks8_ps)

        def emit_A2a2(b, st):
            # matrix stage (depends on the full P accumulation)
            nc.vector.tensor_copy(out=st["p_sb"], in_=st["p_ps"])
            st["p_ps"] = None
            pt_ps = pp_small.tile([128, KC, 66], f16, tag="small",
                                  name=f"pt{b}")
            for k in range(KC):
                nc.tensor.transpose(
                    pt_ps[:, k, 0:66],
                    st["p_sb"][:, 128 * k:128 * (k + 1)],
                    ident[0:65, 0:66],
                )
            nc.vector.tensor_copy(out=st["pt_sb"], in_=pt_ps[:, :, 0:65])
            mat_ps = pp_small.tile([65, 512], f32, tag="small",
                                   name=f"mat_ps{b}")
            for k in range(KC):
                nc.tensor.matmul(
                    mat_ps, st["pt_sb"][:, k, :], wvt[:, k, :],
                    start=(k == 0), stop=(k == KC - 1 and not use_bv),
                    skip_group_check=True,
                )
            if use_bv:
                nc.tensor.matmul(
                    mat_ps, st["ksumn_row"], bv, start=False, stop=True,
                    skip_group_check=True,
                )
            # 1/N folded here (gamma goes into the final psum->sbuf drain)
            nc.vector.tensor_scalar_mul(
                out=st["mat_sb"], in0=mat_ps, scalar1=float(1.0 / N)
            )

        def emit_A2b1(b, st):
            # nq8 finalization -- frees n2q8_ps early for the next batch
            nc.scalar.sqrt(out=st["nq8"], in_=st["n2q8_ps"])
            st["n2q8_ps"] = None

        def emit_A2b2(b, st):
            # s8 + u/tailor chain
            s8_ps = pp_acc.tile([8, 512], f32, tag="acc", name=f"s8{b}")
            for nb in range(NB):
                sl = slice(512 * nb, 512 * (nb + 1))
                nc.tensor.matmul(
                    s8_ps, st["ks8"][:, 8 * nb:8 * (nb + 1)],
                    st["q_raw"][0:64, sl],
                    start=(nb == 0), stop=(nb == NB - 1), skip_group_check=True,
                )
            nq8 = st["nq8"]
            t1 = st["t1"]
            nc.vector.scalar_tensor_tensor(
                out=t1, in0=s8_ps, scalar=float(1.0 / N), in1=nq8,
                op0=alu.mult, op1=alu.add,
            )
            ut = st["ut"]
            nc.vector.reciprocal(out=ut[0:8, :], in_=t1)
            nc.vector.tensor_mul(out=ut[32:40, :], in0=nq8, in1=ut[0:8, :])

        def emit_B_t2(b, st, nb):
            sl = slice(512 * nb, 512 * (nb + 1))
            t2_ps = pp_big.tile([65, 512], f32, tag="big", name=f"t2_{b}_{nb}")
            nc.tensor.matmul(
                t2_ps, selab[:, 65 * nb:65 * (nb + 1)], st["ut"],
                start=True, stop=True,
            )
            qns = work.tile([65, 512], f16, tag="qns", bufs=3,
                            name=f"qns{b}_{nb}")
            nc.vector.tensor_tensor(
                out=qns, in0=st["q_raw"][:, sl], in1=t2_ps, op=alu.mult,
            )
            st["qns"][nb] = qns

        def emit_B_out(b, st, nb, act_only=False, dma_sync=False):
            qns = st["qns"].pop(nb)
            o_sb = outp.tile([128, KC, 512], f16, tag="o", name=f"o_sb{b}_{nb}")
            for cb in range(KC):
                o_ps = pp_big.tile([128, 512], f32, tag="big",
                                   name=f"o_ps{b}_{nb}_{cb}")
                nc.tensor.matmul(
                    o_ps, st["mat_sb"][:, 128 * cb:128 * (cb + 1)], qns,
                    start=True, stop=True,
                )
                # gamma applied during the drain (per-partition scale)
                if act_only or cb % 2 == 0:
                    nc.scalar.activation(
                        out=o_sb[:, cb, :], in_=o_ps, func=act.Copy,
                        scale=gam128,
                    )
                else:
                    nc.vector.tensor_scalar_mul(
                        out=o_sb[:, cb, :], in0=o_ps, scalar1=gam128,
                    )
            if dma_sync == "split":
                nc.sync.dma_start(out=out_d[b, nb, :, 0:2], in_=o_sb[:, 0:2])
                nc.gpsimd.dma_start(out=out_d[b, nb, :, 2:4], in_=o_sb[:, 2:4])
            elif dma_sync:
                nc.sync.dma_start(out=out_d[b, nb], in_=o_sb)
            else:
                nc.gpsimd.dma_start(out=out_d[b, nb], in_=o_sb)

        # ---------------- global pipeline ----------------
        # Slot s of batch b emits (A side):
        #   loads(s+1), QK(s), P(s-2), n2q8(s-1), ktT(s-1) [+chains]
        # Batch b-1's A-side leftovers (P(6), P(7), ktT(7), ...) plus A2a run
        # in batch b slot 0, A2b in slot 1 (freeing its acc-pool psum before
        # batch b's accumulators claim the buffers), and B(b-1) runs shifted:
        # t2(p) at slot p+2, out(p) at slot p+3, finishing in the tail.

        states = {}

        def a_side(b, s):
            st = states[b]
            if s == 0 and 0 not in st["xh"]:
                emit_loads(b, st, 0)
            if s + 1 < NB and s + 1 not in st["xh"]:
                emit_loads(b, st, s + 1)
            if s < NB:
                emit_qk_mm(b, st, s)
            if 2 <= s < NB + 2:
                emit_p_mm(b, st, s - 2)
                emit_n2q8_mm(b, st, s - 2)
            if 1 <= s < NB + 1:
                emit_ktT_mm(b, st, s - 1)
            if s < NB:
                emit_qk_drain(b, st, s)
            if 1 <= s < NB + 1:
                emit_knt_chain(b, st, s - 1)

        def b_side(b, p, act_only=False, dma_sync=False):
            # B phase of batch b at position p (0-based from its start)
            st = states[b]
            if 0 <= p < NB:
                emit_B_t2(b, st, p)
            if 1 <= p < NB + 1:
                emit_B_out(b, st, p - 1, act_only=act_only,
                           dma_sync=dma_sync)

        assert BLOC == 2
        for b in range(BLOC):
            if b not in states:
                states[b] = alloc_state(b)
            for s in range(NB):
                if b > 0 and s == 0:
                    a_side(b - 1, NB)
                    emit_A2a1(b - 1, states[b - 1])
                    a_side(b - 1, NB + 1)
                if b > 0 and s == 1:
                    emit_A2b1(b - 1, states[b - 1])
                    emit_A2b2(b - 1, states[b - 1])
                a_side(b, s)
                if b == 0 and s == 0:
                    nc.sync.dma_start(out=blob[:, CB_IDENT:CBLOB],
                                      in_=blob_d[:, CB_IDENT:CBLOB])
                if b == 0 and s == 1:
                    nc.sync.dma_start(out=blob[:, CB_WVT:CB_IDENT],
                                      in_=blob_d[:, CB_WVT:CB_IDENT])
                if b > 0:
                    if s == 0:
                        emit_A2a2(b - 1, states[b - 1])
                    if s >= 4:
                        b_side(b - 1, s - 4)
                if b + 1 < BLOC and s >= NB - 2:
                    # prefetch the next batch's first loads to avoid a 2MB
                    # DMA burst at the batch boundary
                    if b + 1 not in states:
                        states[b + 1] = alloc_state(b + 1)
                    emit_loads(b + 1, states[b + 1], s - (NB - 2))
        # tail: last batch's A-side leftovers + A2, finish both B phases;
        # batch bl-1's late B chunks keep the PE busy while bl's A2 chains
        # (ksum -> ks8 -> s8 -> u) cross engines; their drains go to ACT so
        # the DVE-side u-chain is never queued behind bulk copies.
        bl = BLOC - 1
        a_side(bl, NB)
        emit_A2a1(bl, states[bl])
        b_side(bl - 1, 4)
        a_side(bl, NB + 1)
        emit_A2a2(bl, states[bl])
        emit_A2b1(bl, states[bl])
        b_side(bl - 1, 5)
        emit_A2b2(bl, states[bl])
        b_side(bl - 1, 6)
        b_side(bl - 1, 7)
        b_side(bl - 1, 8)
        for p in range(NB + 1):
            b_side(bl, p, dma_sync=(p % 2 == 0))

    _split_multi_waits(nc)
    return nc


_CACHE = {}


def _get_module(use_bqk, use_bv):
    key = (use_bqk, use_bv)
    if key not in _CACHE:
        _CACHE[key] = _build_module(*key)
    return _CACHE[key]


def _host_inputs(x, Wq, bq, Wk, bk, Wv, bv, gamma):
    x = np.ascontiguousarray(np.asarray(x, dtype=np.float32)).reshape(B, C, N)
    Wq = np.asarray(Wq, dtype=np.float32)
    Wk = np.asarray(Wk, dtype=np.float32)
    Wv = np.asarray(Wv, dtype=np.float32)
    bq = np.asarray(bq, dtype=np.float32)
    bk = np.asarray(bk, dtype=np.float32)
    bvv = np.asarray(bv, dtype=np.float32)
    gamma = np.asarray(gamma, dtype=np.float32)

    x16 = x.astype(np.float16)
    # x5[b, nb, p, k, c512]: x[b, 128k+p, 512nb+c512]
    x5 = np.ascontiguousarray(
        x16.reshape(B, KC, 128, NB, 512).transpose(0, 3, 2, 1, 4)
    )
    # xt5[b, nb, p, j, c]: x^T[b, 512nb+128j+p, c] = x[b, c, 512nb+128j+p]
    xt5 = np.ascontiguousarray(
        x16.reshape(B, C, NB, 4, 128).transpose(0, 2, 4, 3, 1)
    )

    wqk = np.concatenate([Wq, Wk], axis=0)            # [128, 512]
    wqkt = np.ascontiguousarray(
        wqk.T.reshape(KC, 128, 128).transpose(1, 0, 2)
    ).astype(np.float16)                              # [128, KC, 128]
    wvt = np.ascontiguousarray(
        Wv.T.reshape(KC, 128, 512).transpose(1, 0, 2)
    ).astype(np.float16)                              # [128, KC, 512]

    blob = np.zeros((128, CBLOB), np.float16)
    blob[:, CB_WQKT:CB_WQKT + 512] = wqkt.reshape(128, 512)
    blob[:, CB_WVT:CB_WVT + 2048] = wvt.reshape(128, 2048)
    blob[0:128, CB_IDENT:CB_IDENT + 128] = np.eye(128, dtype=np.float16)
    sel8 = np.zeros((64, 64), np.float16)
    for nb in range(8):
        sel8[:, 8 * nb + nb] = 1.0
    blob[0:64, CB_SEL8:CB_SEL8 + 64] = sel8
    selab = np.zeros((40, 520), np.float16)
    for nb in range(8):
        selab[nb, 65 * nb:65 * nb + 64] = 1.0
        selab[32 + nb, 65 * nb + 64] = 1.0
    blob[0:40, CB_SELAB:CB_SELAB + 520] = selab
    blob[0:128, CB_ONES2:CB_ONES2 + 2] = 1.0
    blob[0:1, CB_ONESN:CB_ONESN + 512] = 1.0
    blob[0:1, CB_BQK:CB_BQK + 128] = np.concatenate([bq, bk]).astype(np.float16)
    blob[0:1, CB_BV:CB_BV + 512] = bvv.astype(np.float16)
    kp = np.zeros((64,), np.float16)
    for nb in range(8):
        kp[8 * nb + nb] = 1.0
    blob[0:1, CB_KPAT:CB_KPAT + 64] = kp

    gam = gamma.reshape(1, 1).astype(np.float32)

    shared = dict(blob=blob, gamma=gam)
    in_maps = []
    for c in range(NCORES):
        m = dict(shared)
        m["x5"] = np.ascontiguousarray(x5[c * BLOC:(c + 1) * BLOC])
        m["xt5"] = np.ascontiguousarray(xt5[c * BLOC:(c + 1) * BLOC])
        in_maps.append(m)
    return in_maps


def run_on_device(in_maps, **kw):
    from concourse.bass_utils import run_bass_kernel_spmd

    m = in_maps[0]
    use_bqk = bool(np.any(m["blob"][0:1, CB_BQK:CB_BQK + 128]))
    use_bv = bool(np.any(m["blob"][0:1, CB_BV:CB_BV + 512]))
    nc = _get_module(use_bqk, use_bv)
    return run_bass_kernel_spmd(nc, in_maps, core_ids=list(range(NCORES)), **kw)


def _assemble(results):
    # o5[b, nb, p, cb, 512] -> out[b, 128cb+p, 512nb+c512]
    outs = []
    for r in results:
        o5 = r["out"].astype(np.float32)        # [BLOC, NB, 128, KC, 512]
        outs.append(o5.transpose(0, 3, 2, 1, 4).reshape(BLOC, C, N))
    return np.concatenate(outs, axis=0)


def kernel(x, Wq, bq, Wk, bk, Wv, bv, gamma):
    in_maps = _host_inputs(x, Wq, bq, Wk, bk, Wv, bv, gamma)
    res = run_on_device(in_maps)
    return _assemble(res.results).reshape(B, C, H, W)


# revision 28
# speedup vs baseline: 1.1701x; 1.1701x over previous
"""Trainium2 Bass kernel for nn_Attention_33638183862624 (linear/Taylor-softmax
attention). Data-parallel over batch: 16 batches -> 8 NeuronCores, 2 each.

Math per batch (C=512, N=4096, CQK=64), x flattened to [C, N]:
  Q = Wq x + bq; K = Wk x + bk           (Q,K: [64, N])
  Qn = Q / ||Q||_col; Kn = K / ||K||_col
  ksum = sum_n Kn[:, n]                  [64]
  denom = N + Qn^T ksum; tailor = 1/denom
  V = Wv x + bv                          (never materialized; fused:)
  matrix = Kn V^T = (Kn x^T) Wv^T + ksum bv^T            [64, 512]
  vsum   = V 1_N  = Wv (x 1_N) + N bv                    [512]
  out[c,n] = gamma * tailor[n] * (vsum[c] + sum_m matrix[m,c] Qn[m,n])

v3 design (fp16, deep software pipeline):
  * All IO fp16; host pre-transposes x^T and lays x/x^T/out out so every DMA
    reads/writes 4KB-contiguous per partition (good SDMA descriptors).
  * All matmuls fp16 (FWL halves LDWEIGHTS; fp32 PSUM accumulate).
  * The PE instruction stream is kept dense (HAM stays at K=8/8): every PE
    consumer trails its cross-engine producer by 1-2 chunks:
      chunk nb emits: QK(nb) | P(nb-2) | n2q8(nb-1) | ktT(nb-1) |
                      t2(b-1) | out(b-1, nb-1)
    and A2 (batch finalization) interleaves with the next batch's chunks.
  * Q stays raw; with s = ksum^T Q_raw and nq = ||Q||_col:
      un := N*u = N/(N*nq + s), and N*tailor = nq*un;
    1/N folded into mat_sb, gamma applied in the final PSUM->SBUF drain.
  * ks8 (the stacked-ksum lhsT for s8) built by one outer-product matmul
    instead of 8 DVE copies.
"""

import numpy as np

B, C, H, W = 16, 512, 64, 64
N = H * W          # 4096
CQK = C // 8       # 64
NCORES = 8
BLOC = B // NCORES  # 2 batches per core
NB = N // 512       # 8 n-chunks of 512
KC = C // 128       # 4 channel chunks of 128


# ---------------------------------------------------------------------------
# Walrus workaround: this container's walrus rejects >1 sync wait per
# instruction ("Too many sync wait commands"). (1) patch the TileContext tail
# drain to carry its waits on single-wait NOPs; (2) post-pass that rewrites
# any instruction with k>1 waits into k-1 single-wait NOPs + the instruction.
# ---------------------------------------------------------------------------

def _apply_tile_patches():
    import concourse.tile as tile
    from concourse import mybir
    from concourse.vector_clock import ScopedClock

    if getattr(tile.TileContext, "_drain_patched", False):
        return

    def _patched_drain_and_barrier(self, tick_clock, wait_clock):
        nop = self.nc.sync.nop(nofuse=True, hint="tail_drain_waits")
        wait_clock.add_sem_waits(
            nop.ins, ScopedClock({None: tick_clock.global_clock})
        )
        si = nop.ins.sync_info
        if si is not None and len(si.on_wait) > 1:
            waits = list(si.on_wait)
            nop.ins.sync_info = mybir.SyncInfo(on_wait=waits[:1], on_update=[])
            rest = waits[1:]
            while rest:
                n2 = self.nc.sync.nop(nofuse=True, hint="tail_drain_waits")
                n2.ins.sync_info = mybir.SyncInfo(on_wait=rest[:1], on_update=[])
                rest = rest[1:]
        self.nc.sync.drain()
        self.nc.all_engine_barrier()
        assert self.sems is not None
        popped = self.nc._tile_sem_poison_stack.pop()
        assert popped is self._sem_poison
        self.nc.clear_and_free_semaphores(list(self.sems.allocated().values()))
        self.nc.all_engine_barrier()

    tile.TileContext._drain_and_barrier = _patched_drain_and_barrier
    tile.TileContext._drain_patched = True


def _split_multi_waits(nc):
    from concourse import mybir

    counter = [0]
    for f in nc.m.functions:
        for bb in f.blocks:
            insts = bb.instructions
            if not any(
                i.sync_info is not None and len(i.sync_info.on_wait) > 1
                for i in insts
            ):
                continue
            new = []
            for ins in insts:
                si = ins.sync_info
                if si is not None and len(si.on_wait) > 1:
                    waits = list(si.on_wait)
                    for w in waits[:-1]:
                        counter[0] += 1
                        nop = mybir.InstNoOp(
                            name=f"I-wsplit-{counter[0]}", ins=[], outs=[]
                        )
                        nop.engine = ins.engine
                        nop.sync_info = mybir.SyncInfo(on_wait=[w], on_update=[])
                        new.append(nop)
                    ins.sync_info = mybir.SyncInfo(
                        on_wait=[waits[-1]], on_update=list(si.on_update)
                    )
                new.append(ins)
            bb.instructions = new


# ---------------------------------------------------------------------------
# Kernel body
# ---------------------------------------------------------------------------

# const blob column layout (fp16, [128, CBLOB]); wqkt/wvt lead so one DMA
# brings everything in with 4.4KB-contiguous per-partition descriptors
CB_WQKT = 0           # [128, 4*128] Wqk^T chunks
CB_WVT = 512          # [128, 4*512] Wv^T chunks
CB_IDENT = 2560       # [128, 128] identity
CB_SEL8 = 2688        # [64, 64] n2q8 one-hot stacker
CB_SELAB = 2752       # [40, 520] u/tailor broadcast selector
CB_ONES2 = 3272       # [128, 2] ones
CB_ONESN = 3274       # [1, 512] ones row
CB_BQK = 3786         # [1, 128] qk bias row
CB_BV = 3914          # [1, 512] v bias row
CB_KPAT = 4426        # [1, 64] ks8 diagonal pattern row
CBLOB = 4490


def _build_module(use_bqk=True, use_bv=True):
    import concourse.bass as bass
    import concourse.tile as tile
    from concourse import mybir

    _apply_tile_patches()
    f16 = mybir.dt.float16
    f32 = mybir.dt.float32
    alu = mybir.AluOpType
    act = mybir.ActivationFunctionType

    nc = bass.Bass("TRN2", target_bir_lowering=False, debug=False)

    # x5:  [b, nb, p, k, 512]  (c = 128k + p)
    # xt5: [b, nb, p, j, 512]  (n = 512nb + 128j + p)
    # o5:  [b, nb, p, cb, 512] (c = 128cb + p)
    x_d = nc.dram_tensor("x5", [BLOC, NB, 128, KC, 512], f16,
                         kind="ExternalInput").ap()
    xt_d = nc.dram_tensor("xt5", [BLOC, NB, 128, 4, 512], f16,
                          kind="ExternalInput").ap()
    blob_d = nc.dram_tensor("blob", [128, CBLOB], f16, kind="ExternalInput").ap()
    gam_d = nc.dram_tensor("gamma", [1, 1], f32, kind="ExternalInput").ap()
    out_d = nc.dram_tensor("out", [BLOC, NB, 128, KC, 512], f16,
                           kind="ExternalOutput").ap()

    from contextlib import ExitStack

    with tile.TileContext(nc) as tc, ExitStack() as ctx, \
            nc.allow_low_precision(reason="fp16 compute validated vs 2e-2 gate"):
        consts = ctx.enter_context(tc.tile_pool(name="consts", bufs=1))
        xpool = ctx.enter_context(tc.tile_pool(name="xpool", bufs=3))
        xtpool = ctx.enter_context(tc.tile_pool(name="xtpool", bufs=5))
        batchp = ctx.enter_context(tc.tile_pool(name="batchp", bufs=2))
        work = ctx.enter_context(tc.tile_pool(name="work", bufs=3))
        outp = ctx.enter_context(tc.tile_pool(name="outp", bufs=4))
        pp_big = ctx.enter_context(tc.tile_pool(name="pp_big", bufs=5, space="PSUM"))
        pp_acc = ctx.enter_context(tc.tile_pool(name="pp_acc", bufs=2, space="PSUM"))
        pp_small = ctx.enter_context(
            tc.tile_pool(name="pp_small", bufs=1, space="PSUM")
        )

        # ---- constants ----
        # wqkt + misc land before the first x loads; the big wvt block is
        # deferred (emitted at batch 0 slot 1) since it's first read at A2a
        blob = consts.tile([128, CBLOB], f16)
        nc.sync.dma_start(out=blob[:, 0:CB_WVT], in_=blob_d[:, 0:CB_WVT])
        gam128 = consts.tile([128, 1], f32)
        nc.sync.dma_start(
            out=gam128,
            in_=bass.AP(
                tensor=gam_d.tensor, offset=gam_d.offset,
                ap=[[0, 128], [1, 1]],
            ),
        )
        wqkt = blob[:, CB_WQKT:CB_WQKT + 512].rearrange(
            "p (k m) -> p k m", m=128)
        wvt = blob[:, CB_WVT:CB_WVT + 2048].rearrange(
            "p (k m) -> p k m", m=512)
        ident = blob[:, CB_IDENT:CB_IDENT + 128]
        sel8 = blob[0:64, CB_SEL8:CB_SEL8 + 64]
        selab = blob[0:40, CB_SELAB:CB_SELAB + 520]
        ones2 = blob[:, CB_ONES2:CB_ONES2 + 2]
        onesn = blob[0:1, CB_ONESN:CB_ONESN + 512]
        bqk = blob[0:1, CB_BQK:CB_BQK + 128]
        bv = blob[0:1, CB_BV:CB_BV + 512]
        kpat = blob[0:1, CB_KPAT:CB_KPAT + 64]

        def alloc_state(b):
            st = {}
            st["q_raw"] = batchp.tile([65, N], f16, tag="q_raw",
                                      name=f"q_raw{b}")
            nc.gpsimd.memset(st["q_raw"][64:65, :], 1.0)
            st["ks8"] = batchp.tile([64, 64], f16, tag="ks8", name=f"ks8_{b}")
            st["ks_parts"] = batchp.tile([65, NB], f32, tag="ks_parts",
                                         name=f"ks_parts{b}")
            st["ksum_h"] = batchp.tile([65, 1], f16, tag="ksum_h",
                                       name=f"ksum_h{b}")
            st["ksumn_row"] = batchp.tile([1, 65], f16, tag="ksumn_row",
                                          name=f"ksumn_row{b}")
            st["p_sb"] = batchp.tile([65, 512], f16, tag="p_sb",
                                     name=f"p_sb{b}")
            st["pt_sb"] = batchp.tile([128, KC, 65], f16, tag="pt_sb",
                                      name=f"pt_sb{b}")
            st["mat_sb"] = batchp.tile([65, 512], f16, tag="mat_sb",
                                       name=f"mat_sb{b}")
            st["ut"] = batchp.tile([40, 512], f16, tag="ut", name=f"ut{b}")
            # rows 8-31 feed the t2 matmul with zero selab weights; stale
            # fp16 NaN patterns would still poison 0*NaN -> zero them.
            nc.gpsimd.memset(st["ut"][0:32, :], 0.0)
            st["nq8"] = batchp.tile([8, 512], f32, tag="nq8", name=f"nq8_{b}")
            st["t1"] = batchp.tile([8, 512], f32, tag="t1", name=f"t1_{b}")
            # persistent 3-deep ring for normalized K^T tiles: the ones
            # column is written once per ring slot instead of per chunk
            # (keeps gpsimd out of the knt critical chain)
            st["knt_ring"] = batchp.tile([128, 3, 4, 65], f16, tag="knt_ring",
                                         name=f"knt_ring{b}")
            nc.gpsimd.memset(st["knt_ring"][:, :, :, 64:65], 1.0)
            st["p_ps"] = None      # lazy (pp_acc)
            st["n2q8_ps"] = None   # lazy (pp_acc)
            st["xh"] = {}
            st["xt"] = {}
            st["k_sb"] = {}
            st["sq"] = {}
            st["knt"] = {}
            st["qns"] = {}
            return st

        # ---------------- emission pieces ----------------

        def emit_loads(b, st, nb, swdge=False):
            eng = nc.gpsimd if swdge else nc.sync
            xh = xpool.tile([128, KC, 512], f16, tag="xh", name=f"xh{b}_{nb}")
            eng.dma_start(out=xh, in_=x_d[b, nb])
            st["xh"][nb] = xh
            xt = xtpool.tile([128, 4, 512], f16, tag="xt", name=f"xt{b}_{nb}")
            eng.dma_start(out=xt, in_=xt_d[b, nb])
            st["xt"][nb] = xt

        def emit_qk_mm(b, st, nb):
            xh = st["xh"].pop(nb)
            qk_ps = pp_big.tile([128, 512], f32, tag="big", name=f"qk{b}_{nb}")
            for k in range(KC):
                nc.tensor.matmul(
                    qk_ps, wqkt[:, k, :], xh[:, k, :],
                    start=(k == 0), stop=(k == KC - 1 and not use_bqk),
                )
            if use_bqk:
                nc.tensor.matmul(qk_ps, bqk, onesn, start=False, stop=True)
            st["qk_ps"] = qk_ps

        def emit_qk_drain(b, st, nb):
            qk_ps = st.pop("qk_ps")
            sl = slice(512 * nb, 512 * (nb + 1))
            nc.vector.tensor_copy(out=st["q_raw"][0:64, sl], in_=qk_ps[0:64, :])
            sq_sb = work.tile([64, 512], f16, tag="sq", bufs=3,
                              name=f"sq{b}_{nb}")
            nc.scalar.square(out=sq_sb, in_=qk_ps[0:64, :])
            st["sq"][nb] = sq_sb
            k_sb = work.tile([64, 512], f16, tag="k_sb", bufs=3,
                             name=f"k_sb{b}_{nb}")
            nc.scalar.copy(out=k_sb, in_=qk_ps[64:128, :])
            st["k_sb"][nb] = k_sb

        def emit_n2q8_mm(b, st, nb):
            sq_sb = st["sq"].pop(nb)
            if st["n2q8_ps"] is None:
                st["n2q8_ps"] = pp_acc.tile([8, 512], f32, tag="acc",
                                            name=f"n2q8_ps{b}")
            nc.tensor.matmul(
                st["n2q8_ps"], sel8[:, 8 * nb:8 * (nb + 1)], sq_sb,
                start=(nb == 0), stop=(nb == NB - 1), skip_group_check=True,
            )

        def emit_ktT_mm(b, st, nb):
            k_sb = st["k_sb"].pop(nb)
            kt_ps = pp_big.tile([128, 4, 64], f16, tag="big",
                                name=f"kt{b}_{nb}")
            for j in range(4):
                nc.tensor.transpose(
                    kt_ps[:, j, :], k_sb[:, 128 * j:128 * (j + 1)],
                    ident[0:64, 0:64],
                )
            st["kt_ps"] = kt_ps

        def emit_knt_chain(b, st, nb):
            kt_ps = st.pop("kt_ps")
            # sum K^2 along the free dim of the transposed tiles:
            # ACT squares straight out of PSUM, one DVE reduce for all 4 j
            ksq = work.tile([128, 4, 64], f16, tag="ksq", bufs=2,
                            name=f"ksq{b}_{nb}")
            nc.scalar.square(out=ksq, in_=kt_ps)
            kn2 = work.tile([128, 4, 1], f32, tag="kn2", bufs=2,
                            name=f"kn2_{b}_{nb}")
            nc.vector.tensor_reduce(
                out=kn2, in_=ksq, axis=mybir.AxisListType.X, op=alu.add,
            )
            nkt = work.tile([128, 4], f32, tag="nkt", bufs=2,
                            name=f"nkt{b}_{nb}")
            nc.scalar.sqrt(out=nkt, in_=kn2[:, :, 0])
            rk = work.tile([128, 4], f32, tag="rk", bufs=2,
                           name=f"rk{b}_{nb}")
            nc.vector.reciprocal(out=rk, in_=nkt)
            knt_sb = st["knt_ring"][:, nb % 3]
            for j in range(4):
                nc.vector.tensor_scalar_mul(
                    out=knt_sb[:, j, 0:64], in0=kt_ps[:, j, :],
                    scalar1=rk[:, j:j + 1],
                )
            st["knt"][nb] = knt_sb
            # ksum partials need only knt -- run a slot ahead of the P stage
            ks_ps = pp_small.tile([65, 2], f32, tag="small", name=f"ksp{b}_{nb}")
            for j in range(4):
                nc.tensor.matmul(
                    ks_ps, knt_sb[:, j, :], ones2,
                    start=(j == 0), stop=(j == 3),
                    skip_group_check=True,
                )
            nc.vector.tensor_copy(
                out=st["ks_parts"][:, nb:nb + 1], in_=ks_ps[:, 0:1]
            )

        def emit_p_mm(b, st, nb):
            knt_sb = st["knt"].pop(nb)
            xt = st["xt"].pop(nb)
            if st["p_ps"] is None:
                st["p_ps"] = pp_acc.tile([65, 512], f32, tag="acc",
                                         name=f"p_ps{b}")
            for j in range(4):
                nc.tensor.matmul(
                    st["p_ps"], knt_sb[:, j, :], xt[:, j, :],
                    start=(nb == 0 and j == 0),
                    stop=(nb == NB - 1 and j == 3),
                    skip_group_check=True,
                )

        def emit_A2a1(b, st):
            # ksum chain + ks8 outer product (depends only on ks_parts)
            nc.vector.reduce_sum(
                out=st["ksum_h"], in_=st["ks_parts"],
                axis=mybir.AxisListType.X,
            )
            ksr_ps = pp_small.tile([1, 66], f32, tag="small", name=f"ksr{b}")
            nc.tensor.matmul(
                ksr_ps, st["ksum_h"], ident[0:65, 0:66], start=True, stop=True,
            )
            nc.vector.tensor_copy(out=st["ksumn_row"], in_=ksr_ps[0:1, 0:65])
            # ks8 = ksum (x) kpat : [64, 64] in one matmul
            ks8_ps = pp_small.tile([64, 64], f32, tag="small", name=f"ks8p{b}")
            nc.tensor.matmul(
                ks8_ps, st["ksumn_row"][0:1, 0:64], kpat, start=True, stop=True,
            )
            nc.scalar.copy(out=st["ks8"], in_=ks8_ps)

        def emit_A2a2(b, st):
            # matrix stage (depends on the full P accumulation)
            nc.vector.tensor_copy(out=st["p_sb"], in_=st["p_ps"])
            st["p_ps"] = None
            pt_ps = pp_small.tile([128, KC, 66], f16, tag="small",
                                  name=f"pt{b}")
            for k in range(KC):
                nc.tensor.transpose(
                    pt_ps[:, k, 0:66],
                    st["p_sb"][:, 128 * k:128 * (k + 1)],
                    ident[0:65, 0:66],
                )
            nc.vector.tensor_copy(out=st["pt_sb"], in_=pt_ps[:, :, 0:65])
            mat_ps = pp_small.tile([65, 512], f32, tag="small",
                                   name=f"mat_ps{b}")
            for k in range(KC):
                nc.tensor.matmul(
                    mat_ps, st["pt_sb"][:, k, :], wvt[:, k, :],
                    start=(k == 0), stop=(k == KC - 1 and not use_bv),
                    skip_group_check=True,
                )
            if use_bv:
                nc.tensor.matmul(
                    mat_ps, st["ksumn_row"], bv, start=False, stop=True,
                    skip_group_check=True,
                )
            # 1/N folded here (gamma goes into the final psum->sbuf drain)
            nc.vector.tensor_scalar_mul(
                out=st["mat_sb"], in0=mat_ps, scalar1=float(1.0 / N)
            )

        def emit_A2b1(b, st):
            # nq8 finalization -- frees n2q8_ps early for the next batch
            nc.scalar.sqrt(out=st["nq8"], in_=st["n2q8_ps"])
            st["n2q8_ps"] = None

        def emit_A2b2(b, st):
            # s8 + u/tailor chain
            s8_ps = pp_acc.tile([8, 512], f32, tag="acc", name=f"s8{b}")
            for nb in range(NB):
                sl = slice(512 * nb, 512 * (nb + 1))
                nc.tensor.matmul(
                    s8_ps, st["ks8"][:, 8 * nb:8 * (nb + 1)],
                    st["q_raw"][0:64, sl],
                    start=(nb == 0), stop=(nb == NB - 1), skip_group_check=True,
                )
            nq8 = st["nq8"]
            t1 = st["t1"]
            nc.vector.scalar_tensor_tensor(
                out=t1, in0=s8_ps, scalar=float(1.0 / N), in1=nq8,
                op0=alu.mult, op1=alu.add,
            )
            ut = st["ut"]
            nc.vector.reciprocal(out=ut[0:8, :], in_=t1)
            nc.vector.tensor_mul(out=ut[32:40, :], in0=nq8, in1=ut[0:8, :])

        def emit_B_t2(b, st, nb):
            sl = slice(512 * nb, 512 * (nb + 1))
            t2_ps = pp_big.tile([65, 512], f32, tag="big", name=f"t2_{b}_{nb}")
            nc.tensor.matmul(
                t2_ps, selab[:, 65 * nb:65 * (nb + 1)], st["ut"],
                start=True, stop=True,
            )
            qns = work.tile([65, 512], f16, tag="qns", bufs=3,
                            name=f"qns{b}_{nb}")
            nc.vector.tensor_tensor(
                out=qns, in0=st["q_raw"][:, sl], in1=t2_ps, op=alu.mult,
            )
            st["qns"][nb] = qns

        def emit_B_out(b, st, nb, act_only=False, dma_sync=False):
            qns = st["qns"].pop(nb)
            o_sb = outp.tile([128, KC, 512], f16, tag="o", name=f"o_sb{b}_{nb}")
            for cb in range(KC):
                o_ps = pp_big.tile([128, 512], f32, tag="big",
                                   name=f"o_ps{b}_{nb}_{cb}")
                nc.tensor.matmul(
                    o_ps, st["mat_sb"][:, 128 * cb:128 * (cb + 1)], qns,
                    start=True, stop=True,
                )
                # gamma applied during the drain (per-partition scale)
                if act_only or cb % 2 == 0:
                    nc.scalar.activation(
                        out=o_sb[:, cb, :], in_=o_ps, func=act.Copy,
                        scale=gam128,
                    )
                else:
                    nc.vector.tensor_scalar_mul(
                        out=o_sb[:, cb, :], in0=o_ps, scalar1=gam128,
                    )
            if dma_sync == "split":
                nc.sync.dma_start(out=out_d[b, nb, :, 0:2], in_=o_sb[:, 0:2])
                nc.gpsimd.dma_start(out=out_d[b, nb, :, 2:4], in_=o_sb[:, 2:4])
            elif dma_sync:
                nc.sync.dma_start(out=out_d[b, nb], in_=o_sb)
            else:
                nc.gpsimd.dma_start(out=out_d[b, nb], in_=o_sb)

        # ---------------- global pipeline ----------------
        # Slot s of batch b emits (A side):
        #   loads(s+1), QK(s), P(s-2), n2q8(s-1), ktT(s-1) [+chains]
        # Batch b-1's A-side leftovers (P(6), P(7), ktT(7), ...) plus A2a run
        # in batch b slot 0, A2b in slot 1 (freeing its acc-pool psum before
        # batch b's accumulators claim the buffers), and B(b-1) runs shifted:
        # t2(p) at slot p+2, out(p) at slot p+3, finishing in the tail.

        states = {}

        def a_side(b, s):
            st = states[b]
            if s == 0 and 0 not in st["xh"]:
                emit_loads(b, st, 0)
            if s + 1 < NB and s + 1 not in st["xh"]:
                emit_loads(b, st, s + 1)
            if s < NB:
                emit_qk_mm(b, st, s)
            if 2 <= s < NB + 2:
                emit_p_mm(b, st, s - 2)
            if 1 <= s < NB + 1:
                emit_n2q8_mm(b, st, s - 1)
                emit_ktT_mm(b, st, s - 1)
            if s < NB:
                emit_qk_drain(b, st, s)
            if 1 <= s < NB + 1:
                emit_knt_chain(b, st, s - 1)

        def b_side(b, p, act_only=False, dma_sync=False):
            # B phase of batch b at position p (0-based from its start)
            st = states[b]
            if 0 <= p < NB:
                emit_B_t2(b, st, p)
            if 1 <= p < NB + 1:
                emit_B_out(b, st, p - 1, act_only=act_only,
                           dma_sync=dma_sync)

        assert BLOC == 2
        for b in range(BLOC):
            if b not in states:
                states[b] = alloc_state(b)
            for s in range(NB):
                if b > 0 and s == 0:
                    a_side(b - 1, NB)
                    emit_A2a1(b - 1, states[b - 1])
                    a_side(b - 1, NB + 1)
                if b > 0 and s == 1:
                    emit_A2b1(b - 1, states[b - 1])
                    emit_A2b2(b - 1, states[b - 1])
                a_side(b, s)
                if b == 0 and s == 0:
                    nc.sync.dma_start(out=blob[:, CB_IDENT:CBLOB],
                                      in_=blob_d[:, CB_IDENT:CBLOB])
                if b == 0 and s == 1:
                    nc.sync.dma_start(out=blob[:, CB_WVT:CB_IDENT],
                                      in_=blob_d[:, CB_WVT:CB_IDENT])
                if b > 0:
                    if s == 0:
                        emit_A2a2(b - 1, states[b - 1])
                    if s >= 4:
                        b_side(b - 1, s - 4)
                if b + 1 < BLOC and s >= NB - 2:
                    # prefetch the next batch's first loads to avoid a 2MB
                    # DMA burst at the batch boundary
                    if b + 1 not in states:
                        states[b + 1] = alloc_state(b + 1)
                    emit_loads(b + 1, states[b + 1], s - (NB - 2))
        # tail: last batch's A-side leftovers + A2, finish both B phases;
        # batch bl-1's late B chunks keep the PE busy while bl's A2 chains
        # (ksum -> ks8 -> s8 -> u) cross engines; their drains go to ACT so
        # the DVE-side u-chain is never queued behind bulk copies.
        bl = BLOC - 1
        a_side(bl, NB)
        emit_A2a1(bl, states[bl])
        b_side(bl - 1, 4)
        a_side(bl, NB + 1)
        emit_A2a2(bl, states[bl])
        b_side(bl - 1, 5)
        emit_A2b1(bl, states[bl])
        b_side(bl - 1, 6)
        emit_A2b2(bl, states[bl])
        b_side(bl - 1, 7)
        b_side(bl - 1, 8)
        for p in range(NB + 1):
            b_side(bl, p, dma_sync=(p % 2 == 0))

    _split_multi_waits(nc)
    return nc


_CACHE = {}


def _get_module(use_bqk, use_bv):
    key = (use_bqk, use_bv)
    if key not in _CACHE:
        _CACHE[key] = _build_module(*key)
    return _CACHE[key]


def _host_inputs(x, Wq, bq, Wk, bk, Wv, bv, gamma):
    x = np.ascontiguousarray(np.asarray(x, dtype=np.float32)).reshape(B, C, N)
    Wq = np.asarray(Wq, dtype=np.float32)
    Wk = np.asarray(Wk, dtype=np.float32)
    Wv = np.asarray(Wv, dtype=np.float32)
    bq = np.asarray(bq, dtype=np.float32)
    bk = np.asarray(bk, dtype=np.float32)
    bvv = np.asarray(bv, dtype=np.float32)
    gamma = np.asarray(gamma, dtype=np.float32)

    x16 = x.astype(np.float16)
    # x5[b, nb, p, k, c512]: x[b, 128k+p, 512nb+c512]
    x5 = np.ascontiguousarray(
        x16.reshape(B, KC, 128, NB, 512).transpose(0, 3, 2, 1, 4)
    )
    # xt5[b, nb, p, j, c]: x^T[b, 512nb+128j+p, c] = x[b, c, 512nb+128j+p]
    xt5 = np.ascontiguousarray(
        x16.reshape(B, C, NB, 4, 128).transpose(0, 2, 4, 3, 1)
    )

    wqk = np.concatenate([Wq, Wk], axis=0)            # [128, 512]
    wqkt = np.ascontiguousarray(
        wqk.T.reshape(KC, 128, 128).transpose(1, 0, 2)
    ).astype(np.float16)                              # [128, KC, 128]
    wvt = np.ascontiguousarray(
        Wv.T.reshape(KC, 128, 512).transpose(1, 0, 2)
    ).astype(np.float16)                              # [128, KC, 512]

    blob = np.zeros((128, CBLOB), np.float16)
    blob[:, CB_WQKT:CB_WQKT + 512] = wqkt.reshape(128, 512)
    blob[:, CB_WVT:CB_WVT + 2048] = wvt.reshape(128, 2048)
    blob[0:128, CB_IDENT:CB_IDENT + 128] = np.eye(128, dtype=np.float16)
    sel8 = np.zeros((64, 64), np.float16)
    for nb in range(8):
        sel8[:, 8 * nb + nb] = 1.0
    blob[0:64, CB_SEL8:CB_SEL8 + 64] = sel8
    selab = np.zeros((40, 520), np.float16)
    for nb in range(8):
        selab[nb, 65 * nb:65 * nb + 64] = 1.0
        selab[32 + nb, 65 * nb + 64] = 1.0
    blob[0:40, CB_SELAB:CB_SELAB + 520] = selab
    blob[0:128, CB_ONES2:CB_ONES2 + 2] = 1.0
    blob[0:1, CB_ONESN:CB_ONESN + 512] = 1.0
    blob[0:1, CB_BQK:CB_BQK + 128] = np.concatenate([bq, bk]).astype(np.float16)
    blob[0:1, CB_BV:CB_BV + 512] = bvv.astype(np.float16)
    kp = np.zeros((64,), np.float16)
    for nb in range(8):
        kp[8 * nb + nb] = 1.0
    blob[0:1, CB_KPAT:CB_KPAT + 64] = kp

    gam = gamma.reshape(1, 1).astype(np.float32)

    shared = dict(blob=blob, gamma=gam)
    in_maps = []
    for c in range(NCORES):
        m = dict(shared)
        m["x5"] = np.ascontiguousarray(x5[c * BLOC:(c + 1) * BLOC])
        m["xt5"] = np.ascontiguousarray(xt5[c * BLOC:(c + 1) * BLOC])
        in_maps.append(m)
    return in_maps


def run_on_device(in_maps, **kw):
    from concourse.bass_utils import run_bass_kernel_spmd

    m = in_maps[0]
    use_bqk = bool(np.any(m["blob"][0:1, CB_BQK:CB_BQK + 128]))
    use_bv = bool(np.any(m["blob"][0:1, CB_BV:CB_BV + 512]))
    nc = _get_module(use_bqk, use_bv)
    return run_bass_kernel_spmd(nc, in_maps, core_ids=list(range(NCORES)), **kw)


def _assemble(results):
    # o5[b, nb, p, cb, 512] -> out[b, 128cb+p, 512nb+c512]
    outs = []
    for r in results:
        o5 = r["out"].astype(np.float32)        # [BLOC, NB, 128, KC, 512]
        outs.append(o5.transpose(0, 3, 2, 1, 4).reshape(BLOC, C, N))
    return np.concatenate(outs, axis=0)


def kernel(x, Wq, bq, Wk, bk, Wv, bv, gamma):
    in_maps = _host_inputs(x, Wq, bq, Wk, bk, Wv, bv, gamma)
    res = run_on_device(in_maps)
    return _assemble(res.results).reshape(B, C, H, W)
